# revision 3
# baseline (speedup 1.0000x reference)
"""Trainium2 Bass kernel for DDN depth-focal loss (nn_DDNLoss).

Data-parallel over batch B=8 across 8 NeuronCores (1 image per core).
Each core computes sum_pixels(weight * focal(depth_logits, target)) for its
image; the host sums the 8 partials and divides by B*H*W.

Layout/staging (host):
  - logits are staged channel-innermost, pixel-partition-major: (128, 240*82)
    with the channel axis PERMUTED per image so that the <=33 channels any
    box target can hit (plus the background bin 80 at position 0) occupy the
    first NSLOT positions. Softmax sums are permutation-invariant, so only the
    gather cares, and its slices become compile-time constants.
  - each box n gets a rasterization key kval = q*64 + pos, where
    q = floor((100 - depth)*1024) orders boxes far-to-near and pos is the
    box's channel position. The host verifies no two overlapping boxes with
    different target bins share a q (tie window ~1e-3 depth units).

Device per core:
  1. Rasterize: per-box row/col interval masks -> PE outer products of
     (row_mask * kval) x col_mask accumulated in PSUM over pixel-disjoint box
     groups; DVE max-combines groups into a (96,320) key z-buffer. The max
     picks the nearest box; its channel position rides in the low 6 bits.
  2. Reshape to (128, 240) via a DRAM bounce, decode pos = key mod 64,
     weight = 13 if covered else 1.
  3. Stream logits; ACT exp (bf16); DVE 3D-reduce over channels -> softmax
     denominator S.
  4. Gather x[target]: init from position 0 (background), then one
     copy_predicated per position with mask (pos == k) - all static slices.
  5. Focal epilogue: logp = x_t - ln S; -alpha*(1-p)^2*logp * weight,
     reduce, gpsimd partition all-reduce, DMA the scalar out.
"""

import numpy as np

import concourse.bacc as bacc
import concourse.bass as bass
import concourse.mybir as mybir
from concourse import bass_isa, tile
from concourse.bass_utils import run_bass_kernel_spmd

# Problem constants (hardcoded per harness contract).
B, C, H, W, N = 8, 81, 96, 320, 32
CP = 82                 # channels padded to even for DVE perf modes
HW = H * W              # 30720 pixels per image
P = 128                 # partitions
J = HW // P             # 240 pixel columns per partition (partition-major)
NSTRIP = 8
JS = J // NSTRIP        # 30 pixel columns per strip
FS = JS * CP            # strip free size

ALPHA, GAMMA = 0.25, 2.0
FG_W, BG_W = 13.0, 1.0
DEPTH_MIN, DEPTH_MAX, NUM_BINS = 0.001, 60.0, 80
BIN_SIZE = 2.0 * (DEPTH_MAX - DEPTH_MIN) / (NUM_BINS * (1 + NUM_BINS))
M_CONST = 100.0         # depth-order key base: q = floor((M - depth)*QSCALE)
QSCALE = 1024.0
GRAN = 64.0             # key = q*GRAN + channel_position
PAD_LOGIT = -20.0       # exp(-20) ~ 2e-9: invisible in the softmax sum

F32 = mybir.dt.float32
BF16 = mybir.dt.bfloat16
I32 = mybir.dt.int32
U8 = mybir.dt.uint8
Alu = mybir.AluOpType
Act = mybir.ActivationFunctionType

_CACHE = {}
LAST_RESULT = [None]


def _build(groups, nslot, stage=99):
    """Build the SPMD single-core program. `groups`: lists of box indices,
    pixel-disjoint within a group on every core (their outer products share a
    PSUM accumulation). `nslot`: number of channel positions the gather scans
    (position 0 = background). `stage` truncates for HW bisection."""
    nc = bacc.Bacc("TRN2", target_bir_lowering=False, debug=True)

    xt = nc.dram_tensor("xt", [P, J * CP], F32, kind="ExternalInput")
    box = nc.dram_tensor("box", [N, 4], F32, kind="ExternalInput")
    kval = nc.dram_tensor("kval", [N, 1], F32, kind="ExternalInput")
    iotw = nc.dram_tensor("iotw", [N, W], F32, kind="ExternalInput")
    ioth = nc.dram_tensor("ioth", [N, H], F32, kind="ExternalInput")
    sgn = nc.dram_tensor("sgn", [N, 4], F32, kind="ExternalInput")
    outv = nc.dram_tensor("outv", [1, 1], F32, kind="ExternalOutput")

    with tile.TileContext(nc) as tc:
        with (
            tc.tile_pool(name="big", bufs=1) as bigp,
            tc.tile_pool(name="est", bufs=3) as estp,
            tc.tile_pool(name="map", bufs=1) as mapp,
            tc.tile_pool(name="msk", bufs=4) as mskp,
            tc.tile_pool(name="sml", bufs=1) as smlp,
            tc.tile_pool(name="drp", bufs=1, space="DRAM") as drp,
            tc.tile_pool(name="ps", bufs=8, space="PSUM") as psp,
        ):
            # ---- small inputs ----
            box_t = smlp.tile([N, 4], F32)
            nc.sync.dma_start(box_t[:], box[:])
            kval_t = smlp.tile([N, 1], F32)
            nc.sync.dma_start(kval_t[:], kval[:])
            iotw_t = smlp.tile([N, W], F32)
            nc.sync.dma_start(iotw_t[:], iotw[:])
            ioth_t = smlp.tile([N, H], F32)
            nc.sync.dma_start(ioth_t[:], ioth[:])
            sgn_t = smlp.tile([N, 4], F32)
            nc.sync.dma_start(sgn_t[:], sgn[:])

            # ---- floor(u1,v1)/ceil(u2,v2) of box corners ----
            # convert rounding is unspecified; fix up against the original:
            # floor = c - (c > x), ceil = c + (c < x).
            bxi = smlp.tile([N, 4], I32)
            nc.vector.tensor_copy(bxi[:], box_t[:])
            bxf = smlp.tile([N, 4], F32)
            nc.vector.tensor_copy(bxf[:], bxi[:])
            dlt = smlp.tile([N, 4], F32)
            nc.vector.tensor_tensor(dlt[:, 0:2], bxf[:, 0:2], box_t[:, 0:2], Alu.is_gt)
            nc.vector.tensor_tensor(dlt[:, 2:4], bxf[:, 2:4], box_t[:, 2:4], Alu.is_lt)
            nc.vector.tensor_tensor(dlt[:], dlt[:], sgn_t[:], Alu.mult)
            nc.vector.tensor_tensor(bxf[:], bxf[:], dlt[:], Alu.add)

            # ---- per-box interval masks over columns / rows ----
            mwa = smlp.tile([N, W], F32)
            nc.vector.tensor_scalar(mwa[:], iotw_t[:], bxf[:, 0:1], None, Alu.is_ge)
            mw = smlp.tile([N, W], F32)
            nc.vector.tensor_scalar(mw[:], iotw_t[:], bxf[:, 2:3], None, Alu.is_lt)
            nc.vector.tensor_tensor(mw[:], mw[:], mwa[:], Alu.mult)

            mha = smlp.tile([N, H], F32)
            nc.vector.tensor_scalar(mha[:], ioth_t[:], bxf[:, 1:2], None, Alu.is_ge)
            mhb = smlp.tile([N, H], F32)
            nc.vector.tensor_scalar(mhb[:], ioth_t[:], bxf[:, 3:4], None, Alu.is_lt)
            mhs = smlp.tile([N, H], F32)
            nc.vector.scalar_tensor_tensor(
                mhs[:], mha[:], kval_t[:, 0:1], mhb[:], Alu.mult, Alu.mult
            )

            # PE requires base partition 0 for operands: flatten the per-box
            # mask rows into single-partition row buffers via sbuf->sbuf DMA.
            mwf = smlp.tile([1, N * W], F32)
            nc.sync.dma_start(mwf[:], mw[:])
            mhsf = smlp.tile([1, N * H], F32)
            nc.sync.dma_start(mhsf[:], mhs[:])

            # ---- rasterize: key z-buffer, max over boxes ----
            zneg = mapp.tile([H, W], F32)
            nc.vector.memset(zneg[:], 0.0)
            for members in groups:
                ps = psp.tile([H, W], F32, tag="ps")
                for i, n in enumerate(members):
                    nc.tensor.matmul(
                        ps[:],
                        mhsf[0:1, n * H : (n + 1) * H],
                        mwf[0:1, n * W : (n + 1) * W],
                        start=(i == 0),
                        stop=(i == len(members) - 1),
                    )
                nc.vector.scalar_tensor_tensor(
                    zneg[:], ps[:], 0.0, zneg[:], Alu.bypass, Alu.max
                )

            # ---- reshape (H,W) -> (P,J) via DRAM bounce (linear order) ----
            zbounce = drp.tile([1, HW], F32)
            nc.sync.dma_start(zbounce[:], zneg[:])
            zmap = mapp.tile([P, J], F32)
            nc.sync.dma_start(zmap[:], zbounce[:])

            if stage == 1:
                nc.sync.dma_start(outv[:], zmap[0:1, 0:1])

            if stage >= 2:
                # ---- decode channel position: pos = key mod GRAN ----
                zh = mapp.tile([P, J], F32)
                nc.vector.tensor_scalar_mul(zh[:], zmap[:], 1.0 / GRAN)
                qi = mapp.tile([P, J], I32)
                nc.vector.tensor_copy(qi[:], zh[:])
                qf = mapp.tile([P, J], F32)
                nc.vector.tensor_copy(qf[:], qi[:])
                qfx = mapp.tile([P, J], F32)
                nc.vector.tensor_tensor(qfx[:], qf[:], zh[:], Alu.is_gt)
                nc.vector.tensor_tensor(qf[:], qf[:], qfx[:], Alu.subtract)
                pos = mapp.tile([P, J], F32)
                nc.vector.scalar_tensor_tensor(
                    pos[:], qf[:], -GRAN, zmap[:], Alu.mult, Alu.add
                )

                wt = mapp.tile([P, J], F32)
                nc.vector.tensor_scalar(
                    wt[:], zmap[:], 0.0, FG_W - BG_W, Alu.is_gt, Alu.mult
                )
                nc.vector.tensor_scalar_add(wt[:], wt[:], BG_W)

                if stage == 2:
                    nc.sync.dma_start(outv[:], pos[0:1, 0:1])

            if stage >= 3:
                # ---- main stream: exp + channel-sum per strip ----
                xs = bigp.tile([P, J * CP], F32)
                sred = mapp.tile([P, J], F32)
                for s in range(NSTRIP):
                    sl = slice(s * FS, (s + 1) * FS)
                    nc.sync.dma_start(xs[:, sl], xt[:, sl])
                    es = estp.tile([P, FS], BF16, tag="es")
                    nc.scalar.activation(es[:], xs[:, sl], Act.Exp)
                    nc.vector.tensor_reduce(
                        sred[:, s * JS : (s + 1) * JS],
                        es[:].rearrange("p (j c) -> p j c", c=CP),
                        axis=mybir.AxisListType.X,
                        op=Alu.add,
                    )
                if stage == 3:
                    nc.sync.dma_start(outv[:], sred[0:1, 0:1])

            if stage >= 4:
                # ---- gather x[target] by channel position (static slices) ----
                xs3 = xs[:].rearrange("p (j c) -> p j c", c=CP)
                gat = mapp.tile([P, J], F32)
                nc.vector.tensor_copy(gat[:], xs3[:, :, 0:1])
                for k in range(1, nslot):
                    mk = mskp.tile([P, J], U8, tag="mk")
                    nc.gpsimd.tensor_scalar(
                        mk[:], pos[:], float(k), None, Alu.is_equal
                    )
                    nc.vector.copy_predicated(gat[:], mk[:], xs3[:, :, k : k + 1])
                if stage == 4:
                    nc.sync.dma_start(outv[:], gat[0:1, 0:1])

            if stage >= 5:
                # ---- focal loss epilogue ----
                lse = mapp.tile([P, J], F32)
                nc.scalar.activation(lse[:], sred[:], Act.Ln)
                logp = mapp.tile([P, J], F32)
                nc.vector.tensor_tensor(logp[:], gat[:], lse[:], Alu.subtract)
                pt = mapp.tile([P, J], F32)
                nc.scalar.activation(pt[:], logp[:], Act.Exp)
                um = mapp.tile([P, J], F32)
                nc.scalar.activation(um[:], pt[:], Act.Identity, scale=-1.0, bias=1.0)
                # tmp = -ALPHA * (1-p)^2
                tmp = mapp.tile([P, J], F32)
                nc.vector.scalar_tensor_tensor(
                    tmp[:], um[:], -ALPHA, um[:], Alu.mult, Alu.mult
                )
                wl = mapp.tile([P, J], F32)
                nc.vector.tensor_tensor(wl[:], logp[:], wt[:], Alu.mult)
                junk = mapp.tile([P, J], F32)
                nc.vector.tensor_tensor(junk[:], tmp[:], wl[:], Alu.mult)
                acc = mapp.tile([P, 1], F32)
                nc.vector.tensor_reduce(
                    acc[:], junk[:], axis=mybir.AxisListType.X, op=Alu.add
                )
                tot = mapp.tile([P, 1], F32)
                nc.gpsimd.partition_all_reduce(
                    tot[:], acc[:], channels=P, reduce_op=bass_isa.ReduceOp.add
                )
                nc.sync.dma_start(outv[:], tot[0:1, 0:1])

    nc.finalize()
    return nc


def _group_boxes(bxs_all):
    """Greedy-pack boxes into pixel-disjoint groups, uniform across cores."""
    def overlap(a, b):
        return not (
            a[2] <= b[0] or b[2] <= a[0] or a[3] <= b[1] or b[3] <= a[1]
        )

    groups = []
    for n in range(N):
        placed = False
        for g in groups:
            ok = True
            for m in g:
                for bimg in bxs_all:
                    if overlap(bimg[n], bimg[m]):
                        ok = False
                        break
                if not ok:
                    break
            if ok:
                g.append(n)
                placed = True
                break
        if not placed:
            groups.append([n])
    return groups


def _ref_bin(d):
    """Per-box target bin, replicating the reference's float32 LID binning."""
    d = np.float32(d)
    a = np.float32(1.0) + np.float32(8.0) * (d - np.float32(DEPTH_MIN)) / np.float32(
        BIN_SIZE
    )
    idx = np.float32(-0.5) + np.float32(0.5) * np.sqrt(a, dtype=np.float32)
    return int(np.int32(idx))    # trunc toward zero; idx >= 0 here


def _host_prep(depth_logits, gt_boxes2d, gt_center_depth):
    """Stage per-core inputs: permuted channel layout, raster keys, grouping."""
    xt = np.transpose(depth_logits, (0, 2, 3, 1)).reshape(B, HW, C)
    boxes = gt_boxes2d.reshape(B, N, 4)
    depths = gt_center_depth.reshape(B, N)

    fbox = np.concatenate(
        [np.floor(boxes[:, :, :2]), np.ceil(boxes[:, :, 2:])], axis=2
    )
    groups = _group_boxes(fbox)

    def overlap(a, b):
        return not (
            a[2] <= b[0] or b[2] <= a[0] or a[3] <= b[1] or b[3] <= a[1]
        )

    bins = np.array(
        [[_ref_bin(depths[b, n]) for n in range(N)] for b in range(B)], np.int32
    )
    q = np.floor((M_CONST - depths.astype(np.float64)) * QSCALE).astype(np.int64)

    # Safety: overlapping boxes with different target bins must not share a
    # quantized depth, else the key max could pick the wrong channel.
    for b in range(B):
        for i in range(N):
            for jj in range(i + 1, N):
                if (
                    overlap(fbox[b, i], fbox[b, jj])
                    and bins[b, i] != bins[b, jj]
                    and q[b, i] == q[b, jj]
                ):
                    raise RuntimeError(
                        f"depth-quantization tie: img {b} boxes {i},{jj}"
                    )

    chans_per_core = []
    for b in range(B):
        chans = [NUM_BINS] + sorted(set(bins[b].tolist()))
        chans_per_core.append(chans)
    nslot = max(len(c) for c in chans_per_core)
    assert nslot <= GRAN

    xtp = np.empty((B, P, J * CP), np.float32)
    kval = np.zeros((B, N, 1), np.float32)
    for b in range(B):
        chans = chans_per_core[b]
        pos_of = {ch: i for i, ch in enumerate(chans)}
        rest = [ch for ch in range(C) if ch not in pos_of]
        perm = chans + rest
        assert len(perm) == C
        xb = np.full((HW, CP), PAD_LOGIT, np.float32)
        xb[:, :C] = xt[b][:, perm]
        xtp[b] = xb.reshape(P, J * CP)
        for n in range(N):
            kval[b, n, 0] = np.float32(q[b, n] * int(GRAN) + pos_of[bins[b, n]])

    return xtp, boxes, kval, groups, nslot


def kernel(depth_logits, gt_boxes2d, gt_boxes3d, gt_center_depth, num_gt_per_img):
    depth_logits = np.asarray(depth_logits, dtype=np.float32)
    gt_boxes2d = np.asarray(gt_boxes2d, dtype=np.float32)
    gt_center_depth = np.asarray(gt_center_depth, dtype=np.float32)

    xtp, boxes, kval, groups, nslot = _host_prep(
        depth_logits, gt_boxes2d, gt_center_depth
    )

    key = (tuple(tuple(g) for g in groups), nslot)
    if key not in _CACHE:
        _CACHE[key] = _build([list(g) for g in groups], nslot)
    nc = _CACHE[key]

    iotw = np.broadcast_to(np.arange(W, dtype=np.float32), (N, W)).copy()
    ioth = np.broadcast_to(np.arange(H, dtype=np.float32), (N, H)).copy()
    sgn = np.broadcast_to(
        np.array([-1.0, -1.0, 1.0, 1.0], dtype=np.float32), (N, 4)
    ).copy()

    in_maps = []
    for b in range(B):
        in_maps.append(
            {
                "xt": np.ascontiguousarray(xtp[b]),
                "box": np.ascontiguousarray(boxes[b]),
                "kval": np.ascontiguousarray(kval[b]),
                "iotw": iotw,
                "ioth": ioth,
                "sgn": sgn,
            }
        )

    res = run_bass_kernel_spmd(nc, in_maps, core_ids=list(range(B)))
    LAST_RESULT[0] = res
    total = 0.0
    for b in range(B):
        total += float(res.results[b]["outv"][0, 0])
    return np.float32(total / (B * H * W))

